# revision 15
# baseline (speedup 1.0000x reference)
"""Trainium2 Bass kernel for nn_ExpertLayer (MoE with top-1 routing).

Strategy
--------
The reference computes every expert densely but the output only uses, per
token, the expert selected by argmax(softmax(gate)) — so only that expert's
MLP affects the result.  We exploit that sparsity (8x FLOP reduction) and
shard expert-parallel: core e owns expert e's weights (w1[e], b1[e], w2[e],
b2[e]) and processes exactly the tokens routed to expert e.

Host (fp32, exact routing):
  gate logits -> softmax -> argmax, balance loss, group tokens by expert,
  pad each group to a common capacity C, transpose to feature-major.
Device (per core, bf16 matmuls with fp32 PSUM accumulation):
  hT = relu(w1.T @ xT + b1)   [1024, C]
  yT = w2.T @ hT + b2         [512, C]
  zT = proj_w.T @ yT + proj_b [512, C]  (fp32 out)
Everything stays feature-major ([feature, token]) so no on-device
transposes are needed; the host transposes in/out (cheap numpy).
Host then scatters rows back to the original token order.
"""

import numpy as np
import ml_dtypes

import concourse.bass as bass  # noqa: F401  (namespace init)
import concourse.mybir as mybir
import concourse.tile as tile
from concourse import bacc
from concourse.bass_utils import run_bass_kernel_spmd

E = 8          # experts == cores
D = 512        # d_model
H = 1024       # expert hidden
N_CORES = 8
BALANCE_COEF = 0.01

BF16 = ml_dtypes.bfloat16

# test.py can flip this to capture an NTFF profile of the SPMD run.
TRACE = False
LAST_EXEC_NS = None
LAST_MEAN_EXEC_NS = None


def _route_host(flat, gate_w, gate_b):
    """fp32 gate + softmax + argmax + balance loss, mirroring the reference.

    Uses jax on CPU when available so the numerics match the jax reference
    bit-for-bit; falls back to numpy (argmax-equivalent in fp32).
    """
    T = flat.shape[0]
    try:
        import jax
        import jax.numpy as jnp

        cpu = jax.devices("cpu")[0]
        with jax.default_device(cpu):
            logits = jnp.asarray(flat) @ jnp.asarray(gate_w) + jnp.asarray(gate_b)
            rw = jax.nn.softmax(logits, axis=-1)
            choice = jnp.argmax(rw, axis=-1)
            counts = jnp.zeros((E,), jnp.float32).at[choice].add(1.0)
            probs = counts / T
            loss = -jnp.sum(probs * jnp.log(probs + 1e-10)) * BALANCE_COEF
            return np.asarray(choice), np.float32(loss)
    except Exception:
        logits = flat @ gate_w + gate_b[None, :]
        m = logits.max(axis=-1, keepdims=True)
        ex = np.exp(logits - m)
        rw = ex / ex.sum(axis=-1, keepdims=True)
        choice = np.argmax(rw, axis=-1)
        counts = np.bincount(choice, minlength=E).astype(np.float32)
        probs = counts / np.float32(T)
        loss = np.float32(
            -np.sum(probs * np.log(probs + np.float32(1e-10))) * np.float32(BALANCE_COEF)
        )
        return choice, loss


def _build(C, chunks):
    """Bass program for one core: expert MLP + projection over C tokens."""
    nc = bacc.Bacc("TRN2", target_bir_lowering=False)
    bf16, f32 = mybir.dt.bfloat16, mybir.dt.float32

    xT = nc.dram_tensor("xT", [D, C], bf16, kind="ExternalInput")
    w1 = nc.dram_tensor("w1", [D, H], bf16, kind="ExternalInput")
    w2 = nc.dram_tensor("w2", [H, D], bf16, kind="ExternalInput")
    pw = nc.dram_tensor("pw", [D, D], bf16, kind="ExternalInput")
    b1 = nc.dram_tensor("b1", [128, H // 128], f32, kind="ExternalInput")
    b2 = nc.dram_tensor("b2", [128, D // 128], f32, kind="ExternalInput")
    pb = nc.dram_tensor("pb", [128, D // 128], f32, kind="ExternalInput")
    outT = nc.dram_tensor("outT", [D, C], f32, kind="ExternalOutput")

    Add, Max = mybir.AluOpType.add, mybir.AluOpType.max
    ND, NH = D // 128, H // 128  # 4, 8

    with tile.TileContext(nc) as tc:
        with (
            tc.tile_pool(name="const", bufs=1) as cpool,
            tc.tile_pool(name="work", bufs=3) as wpool,
            tc.tile_pool(name="zout", bufs=3) as zpool,
            tc.tile_pool(name="psum", bufs=2, space="PSUM") as ppool,
        ):
            x_sb = [cpool.tile([128, C], bf16, tag=f"x{dc}", name=f"x{dc}") for dc in range(ND)]
            w1_sb = [cpool.tile([128, H], bf16, tag=f"w1_{dc}", name=f"w1_{dc}") for dc in range(ND)]
            w2_sb = [cpool.tile([128, D], bf16, tag=f"w2_{hc}", name=f"w2_{hc}") for hc in range(NH)]
            pw_sb = [cpool.tile([128, D], bf16, tag=f"pw_{dc}", name=f"pw_{dc}") for dc in range(ND)]
            b1_sb = cpool.tile([128, NH], f32, tag="b1")
            b2_sb = cpool.tile([128, ND], f32, tag="b2")
            pb_sb = cpool.tile([128, ND], f32, tag="pb")

            # PE pre-warm: dummy matmuls on a zeroed tile run while the input
            # DMAs stream in, so the HAM clock-gate is at 8/8 when the real
            # matmul stream starts (saves ~4-6us of cold-rate matmuls).
            warm = cpool.tile([128, 512], bf16, tag="warm")
            nc.vector.memset(warm[:], 0)
            for _ in range(12):
                pwarm = ppool.tile([128, 512], mybir.dt.float32, tag="pz", name="pwarm")
                nc.tensor.matmul(
                    pwarm[:, :256], warm[:, :128], warm[:, :256], start=True, stop=True
                )

            # Input DMAs in critical-path order, issue split across the two
            # HWDGE-capable engines (SP + ACT) — each dma_start costs ~600ns
            # of issue time, so the w1 + x-chunk-0 critical path halves.
            # Non-critical x columns load as one big DMA per row-block.
            qs = [nc.sync, nc.scalar]
            c0 = chunks[0]
            for dc in range(ND):
                qs[dc % 2].dma_start(w1_sb[dc][:], w1[dc * 128 : (dc + 1) * 128, :])
            for dc in range(ND):
                qs[dc % 2].dma_start(
                    x_sb[dc][:, 0:c0], xT[dc * 128 : (dc + 1) * 128, 0:c0]
                )
            nc.scalar.dma_start(b1_sb[:], b1[:])
            nc.sync.dma_start(b2_sb[:], b2[:])
            nc.sync.dma_start(pb_sb[:], pb[:])
            for dc in range(ND):
                nc.sync.dma_start(
                    x_sb[dc][:, c0:C], xT[dc * 128 : (dc + 1) * 128, c0:C]
                )
            for hc in range(NH):
                nc.sync.dma_start(w2_sb[hc][:], w2[hc * 128 : (hc + 1) * 128, :])
            for dc in range(ND):
                nc.sync.dma_start(pw_sb[dc][:], pw[dc * 128 : (dc + 1) * 128, :])

            off = 0
            for tn in chunks:
                # hT[hc] = relu(w1.T @ x + b1)  -- one 128-row H-chunk at a time
                h_sb = []
                for hc in range(NH):
                    ph = ppool.tile([128, 512], mybir.dt.float32, tag="ph", name="ph", bufs=3)
                    for dc in range(ND):
                        nc.tensor.matmul(
                            ph[:, :tn],
                            w1_sb[dc][:, hc * 128 : (hc + 1) * 128],
                            x_sb[dc][:, off : off + tn],
                            start=(dc == 0),
                            stop=(dc == ND - 1),
                        )
                    h = wpool.tile([128, 512], mybir.dt.bfloat16, tag=f"h{hc}", name=f"h{hc}")
                    if hc % 2 == 0:
                        nc.vector.tensor_scalar(
                            h[:, :tn], ph[:, :tn], b1_sb[:, hc : hc + 1], 0.0, Add, Max
                        )
                    else:
                        nc.scalar.activation(
                            h[:, :tn], ph[:, :tn],
                            mybir.ActivationFunctionType.Relu,
                            bias=b1_sb[:, hc : hc + 1],
                        )
                    h_sb.append(h)

                # yT[dc] = w2.T @ h + b2
                y_sb = []
                for dc in range(ND):
                    py = ppool.tile([128, 512], mybir.dt.float32, tag="py", name="py", bufs=3)
                    for hc in range(NH):
                        nc.tensor.matmul(
                            py[:, :tn],
                            w2_sb[hc][:, dc * 128 : (dc + 1) * 128],
                            h_sb[hc][:, :tn],
                            start=(hc == 0),
                            stop=(hc == NH - 1),
                        )
                    y = wpool.tile([128, 512], mybir.dt.bfloat16, tag=f"y{dc}", name=f"y{dc}")
                    nc.vector.tensor_scalar_add(y[:, :tn], py[:, :tn], b2_sb[:, dc : dc + 1])
                    y_sb.append(y)

                # zT[do] = proj_w.T @ y + proj_b  (fp32 out) -> DRAM
                for do in range(ND):
                    pz = ppool.tile([128, 512], mybir.dt.float32, tag="pz", name="pz")
                    for di in range(ND):
                        nc.tensor.matmul(
                            pz[:, :tn],
                            pw_sb[di][:, do * 128 : (do + 1) * 128],
                            y_sb[di][:, :tn],
                            start=(di == 0),
                            stop=(di == ND - 1),
                        )
                    z = zpool.tile([128, 512], mybir.dt.float32, tag="z", name="z")
                    nc.scalar.activation(
                        z[:, :tn], pz[:, :tn],
                        mybir.ActivationFunctionType.Identity,
                        bias=pb_sb[:, do : do + 1],
                    )
                    nc.sync.dma_start(
                        outT[do * 128 : (do + 1) * 128, off : off + tn], z[:, :tn]
                    )
                off += tn

    nc.finalize()
    return nc


def kernel(x, gate_w, gate_b, w1, b1, w2, b2, proj_w, proj_b):
    global LAST_EXEC_NS, LAST_MEAN_EXEC_NS
    x = np.asarray(x, dtype=np.float32)
    gate_w = np.asarray(gate_w, dtype=np.float32)
    gate_b = np.asarray(gate_b, dtype=np.float32)
    w1 = np.asarray(w1, dtype=np.float32)
    b1 = np.asarray(b1, dtype=np.float32)
    w2 = np.asarray(w2, dtype=np.float32)
    b2 = np.asarray(b2, dtype=np.float32)
    proj_w = np.asarray(proj_w, dtype=np.float32)
    proj_b = np.asarray(proj_b, dtype=np.float32)

    B, S, d_model = x.shape
    T = B * S
    flat = np.ascontiguousarray(x.reshape(T, d_model))

    choice, balance_loss = _route_host(flat, gate_w, gate_b)
    idx = [np.nonzero(choice == e)[0] for e in range(E)]
    maxc = max(len(i) for i in idx)
    C = max(((maxc + 127) // 128) * 128, 128)
    # equal-size chunks (<=512 for one PSUM bank) keep every matmul in the
    # regime where LDWEIGHTS hides under the previous matmul
    n_chunks = -(-C // 512)
    base = C // n_chunks
    rem = C - base * n_chunks
    chunks = [base + (1 if i < rem else 0) for i in range(n_chunks)]

    nc = _build(C, chunks)

    pw_b = proj_w.astype(BF16)
    pb_h = np.ascontiguousarray(proj_b.reshape(D // 128, 128).T)
    in_maps = []
    for e in range(E):
        xg = np.zeros((C, d_model), np.float32)
        xg[: len(idx[e])] = flat[idx[e]]
        in_maps.append(
            {
                "xT": np.ascontiguousarray(xg.T).astype(BF16),
                "w1": w1[e].astype(BF16),
                "w2": w2[e].astype(BF16),
                "pw": pw_b,
                "b1": np.ascontiguousarray(b1[e].reshape(H // 128, 128).T),
                "b2": np.ascontiguousarray(b2[e].reshape(D // 128, 128).T),
                "pb": pb_h,
            }
        )

    kwargs = {}
    if TRACE:
        kwargs = {"trace": True, "trace_cores": list(range(N_CORES))}
    res = run_bass_kernel_spmd(nc, in_maps, core_ids=list(range(N_CORES)), **kwargs)
    LAST_EXEC_NS = res.exec_time_ns
    LAST_MEAN_EXEC_NS = res.mean_exec_time_ns

    out = np.empty((T, d_model), np.float32)
    for e in range(E):
        n = len(idx[e])
        if n:
            out[idx[e]] = res.results[e]["outT"].T[:n]
    return out.reshape(B, S, d_model), balance_loss


# revision 16
# speedup vs baseline: 1.0279x; 1.0279x over previous
"""Trainium2 Bass kernel for nn_ExpertLayer (MoE with top-1 routing).

Strategy
--------
The reference computes every expert densely but the output only uses, per
token, the expert selected by argmax(softmax(gate)) — so only that expert's
MLP affects the result.  We exploit that sparsity (8x FLOP reduction) and
shard expert-parallel: core e owns expert e's weights (w1[e], b1[e], w2[e],
b2[e]) and processes exactly the tokens routed to expert e.

Host (fp32, exact routing):
  gate logits -> softmax -> argmax, balance loss, group tokens by expert,
  pad each group to a common capacity C, transpose to feature-major.
Device (per core, bf16 matmuls with fp32 PSUM accumulation):
  hT = relu(w1.T @ xT + b1)   [1024, C]
  yT = w2.T @ hT + b2         [512, C]
  zT = proj_w.T @ yT + proj_b [512, C]  (fp32 out)
Everything stays feature-major ([feature, token]) so no on-device
transposes are needed; the host transposes in/out (cheap numpy).
Host then scatters rows back to the original token order.
"""

import numpy as np
import ml_dtypes

import concourse.bass as bass  # noqa: F401  (namespace init)
import concourse.mybir as mybir
import concourse.tile as tile
from concourse import bacc
from concourse.bass_utils import run_bass_kernel_spmd

E = 8          # experts == cores
D = 512        # d_model
H = 1024       # expert hidden
N_CORES = 8
BALANCE_COEF = 0.01

BF16 = ml_dtypes.bfloat16

# test.py can flip this to capture an NTFF profile of the SPMD run.
TRACE = False
LAST_EXEC_NS = None
LAST_MEAN_EXEC_NS = None


def _route_host(flat, gate_w, gate_b):
    """fp32 gate + softmax + argmax + balance loss, mirroring the reference.

    Uses jax on CPU when available so the numerics match the jax reference
    bit-for-bit; falls back to numpy (argmax-equivalent in fp32).
    """
    T = flat.shape[0]
    try:
        import jax
        import jax.numpy as jnp

        cpu = jax.devices("cpu")[0]
        with jax.default_device(cpu):
            logits = jnp.asarray(flat) @ jnp.asarray(gate_w) + jnp.asarray(gate_b)
            rw = jax.nn.softmax(logits, axis=-1)
            choice = jnp.argmax(rw, axis=-1)
            counts = jnp.zeros((E,), jnp.float32).at[choice].add(1.0)
            probs = counts / T
            loss = -jnp.sum(probs * jnp.log(probs + 1e-10)) * BALANCE_COEF
            return np.asarray(choice), np.float32(loss)
    except Exception:
        logits = flat @ gate_w + gate_b[None, :]
        m = logits.max(axis=-1, keepdims=True)
        ex = np.exp(logits - m)
        rw = ex / ex.sum(axis=-1, keepdims=True)
        choice = np.argmax(rw, axis=-1)
        counts = np.bincount(choice, minlength=E).astype(np.float32)
        probs = counts / np.float32(T)
        loss = np.float32(
            -np.sum(probs * np.log(probs + np.float32(1e-10))) * np.float32(BALANCE_COEF)
        )
        return choice, loss


def _build(C, chunks):
    """Bass program for one core: expert MLP + projection over C tokens."""
    nc = bacc.Bacc("TRN2", target_bir_lowering=False)
    bf16, f32 = mybir.dt.bfloat16, mybir.dt.float32

    xT = nc.dram_tensor("xT", [D, C], bf16, kind="ExternalInput")
    w1 = nc.dram_tensor("w1", [D, H], bf16, kind="ExternalInput")
    w2 = nc.dram_tensor("w2", [H, D], bf16, kind="ExternalInput")
    pw = nc.dram_tensor("pw", [D, D], bf16, kind="ExternalInput")
    b1 = nc.dram_tensor("b1", [128, H // 128], f32, kind="ExternalInput")
    b2 = nc.dram_tensor("b2", [128, D // 128], f32, kind="ExternalInput")
    pb = nc.dram_tensor("pb", [128, D // 128], f32, kind="ExternalInput")
    outT = nc.dram_tensor("outT", [D, C], f32, kind="ExternalOutput")

    Add, Max = mybir.AluOpType.add, mybir.AluOpType.max
    ND, NH = D // 128, H // 128  # 4, 8

    with tile.TileContext(nc) as tc:
        with (
            tc.tile_pool(name="const", bufs=1) as cpool,
            tc.tile_pool(name="work", bufs=2) as wpool,
            tc.tile_pool(name="zout", bufs=3) as zpool,
            tc.tile_pool(name="psum", bufs=2, space="PSUM") as ppool,
        ):
            x_sb = [cpool.tile([128, C], bf16, tag=f"x{dc}", name=f"x{dc}") for dc in range(ND)]
            w1_sb = [cpool.tile([128, H], bf16, tag=f"w1_{dc}", name=f"w1_{dc}") for dc in range(ND)]
            w2_sb = [cpool.tile([128, D], bf16, tag=f"w2_{hc}", name=f"w2_{hc}") for hc in range(NH)]
            pw_sb = [cpool.tile([128, D], bf16, tag=f"pw_{dc}", name=f"pw_{dc}") for dc in range(ND)]
            b1_sb = cpool.tile([128, NH], f32, tag="b1")
            b2_sb = cpool.tile([128, ND], f32, tag="b2")
            pb_sb = cpool.tile([128, ND], f32, tag="pb")

            # PE pre-warm: dummy matmuls on a zeroed tile run while the input
            # DMAs stream in, so the HAM clock-gate is at 8/8 when the real
            # matmul stream starts (saves ~4-6us of cold-rate matmuls).
            warm = cpool.tile([128, 512], bf16, tag="warm")
            nc.vector.memset(warm[:], 0)
            for _ in range(12):
                pwarm = ppool.tile([128, 512], mybir.dt.float32, tag="pz", name="pwarm")
                nc.tensor.matmul(
                    pwarm[:, :256], warm[:, :128], warm[:, :256], start=True, stop=True
                )

            # Input DMAs in critical-path order, issue split across the two
            # HWDGE-capable engines (SP + ACT) — each dma_start costs ~600ns
            # of issue time, so the w1 + x-chunk-0 critical path halves.
            # Non-critical x columns load as one big DMA per row-block.
            qs = [nc.sync, nc.scalar]
            c0 = chunks[0]
            for dc in range(ND):
                qs[dc % 2].dma_start(w1_sb[dc][:], w1[dc * 128 : (dc + 1) * 128, :])
            for dc in range(ND):
                qs[dc % 2].dma_start(
                    x_sb[dc][:, 0:c0], xT[dc * 128 : (dc + 1) * 128, 0:c0]
                )
            nc.scalar.dma_start(b1_sb[:], b1[:])
            nc.sync.dma_start(b2_sb[:], b2[:])
            nc.sync.dma_start(pb_sb[:], pb[:])
            for dc in range(ND):
                nc.sync.dma_start(
                    x_sb[dc][:, c0:C], xT[dc * 128 : (dc + 1) * 128, c0:C]
                )
            for hc in range(NH):
                nc.sync.dma_start(w2_sb[hc][:], w2[hc * 128 : (hc + 1) * 128, :])
            for dc in range(ND):
                nc.sync.dma_start(pw_sb[dc][:], pw[dc * 128 : (dc + 1) * 128, :])

            off = 0
            for tn in chunks:
                # hT[hc] = relu(w1.T @ x + b1)  -- one 128-row H-chunk at a time
                h_sb = []
                for hc in range(NH):
                    ph = ppool.tile([128, 512], mybir.dt.float32, tag="ph", name="ph", bufs=3)
                    for dc in range(ND):
                        nc.tensor.matmul(
                            ph[:, :tn],
                            w1_sb[dc][:, hc * 128 : (hc + 1) * 128],
                            x_sb[dc][:, off : off + tn],
                            start=(dc == 0),
                            stop=(dc == ND - 1),
                        )
                    h = wpool.tile([128, 512], mybir.dt.bfloat16, tag=f"h{hc}", name=f"h{hc}")
                    if hc % 2 == 0:
                        nc.vector.tensor_scalar(
                            h[:, :tn], ph[:, :tn], b1_sb[:, hc : hc + 1], 0.0, Add, Max
                        )
                    else:
                        nc.scalar.activation(
                            h[:, :tn], ph[:, :tn],
                            mybir.ActivationFunctionType.Relu,
                            bias=b1_sb[:, hc : hc + 1],
                        )
                    h_sb.append(h)

                # yT[dc] = w2.T @ h + b2
                y_sb = []
                for dc in range(ND):
                    py = ppool.tile([128, 512], mybir.dt.float32, tag="py", name="py", bufs=3)
                    for hc in range(NH):
                        nc.tensor.matmul(
                            py[:, :tn],
                            w2_sb[hc][:, dc * 128 : (dc + 1) * 128],
                            h_sb[hc][:, :tn],
                            start=(hc == 0),
                            stop=(hc == NH - 1),
                        )
                    y = wpool.tile([128, 512], mybir.dt.bfloat16, tag=f"y{dc}", name=f"y{dc}")
                    nc.vector.tensor_scalar_add(y[:, :tn], py[:, :tn], b2_sb[:, dc : dc + 1])
                    y_sb.append(y)

                # zT[do] = proj_w.T @ y + proj_b  (fp32 out) -> DRAM
                for do in range(ND):
                    pz = ppool.tile([128, 512], mybir.dt.float32, tag="pz", name="pz")
                    for di in range(ND):
                        nc.tensor.matmul(
                            pz[:, :tn],
                            pw_sb[di][:, do * 128 : (do + 1) * 128],
                            y_sb[di][:, :tn],
                            start=(di == 0),
                            stop=(di == ND - 1),
                        )
                    z = zpool.tile([128, 512], mybir.dt.float32, tag="z", name="z")
                    nc.scalar.activation(
                        z[:, :tn], pz[:, :tn],
                        mybir.ActivationFunctionType.Identity,
                        bias=pb_sb[:, do : do + 1],
                    )
                    nc.sync.dma_start(
                        outT[do * 128 : (do + 1) * 128, off : off + tn], z[:, :tn]
                    )
                off += tn

    nc.finalize()
    return nc


def kernel(x, gate_w, gate_b, w1, b1, w2, b2, proj_w, proj_b):
    global LAST_EXEC_NS, LAST_MEAN_EXEC_NS
    x = np.asarray(x, dtype=np.float32)
    gate_w = np.asarray(gate_w, dtype=np.float32)
    gate_b = np.asarray(gate_b, dtype=np.float32)
    w1 = np.asarray(w1, dtype=np.float32)
    b1 = np.asarray(b1, dtype=np.float32)
    w2 = np.asarray(w2, dtype=np.float32)
    b2 = np.asarray(b2, dtype=np.float32)
    proj_w = np.asarray(proj_w, dtype=np.float32)
    proj_b = np.asarray(proj_b, dtype=np.float32)

    B, S, d_model = x.shape
    T = B * S
    flat = np.ascontiguousarray(x.reshape(T, d_model))

    choice, balance_loss = _route_host(flat, gate_w, gate_b)
    idx = [np.nonzero(choice == e)[0] for e in range(E)]
    maxc = max(len(i) for i in idx)
    C = max(((maxc + 127) // 128) * 128, 128)
    # equal-size chunks (<=512 for one PSUM bank) keep every matmul in the
    # regime where LDWEIGHTS hides under the previous matmul
    n_chunks = -(-C // 512)
    base = C // n_chunks
    rem = C - base * n_chunks
    chunks = [base + (1 if i < rem else 0) for i in range(n_chunks)]

    nc = _build(C, chunks)

    pw_b = proj_w.astype(BF16)
    pb_h = np.ascontiguousarray(proj_b.reshape(D // 128, 128).T)
    in_maps = []
    for e in range(E):
        xg = np.zeros((C, d_model), np.float32)
        xg[: len(idx[e])] = flat[idx[e]]
        in_maps.append(
            {
                "xT": np.ascontiguousarray(xg.T).astype(BF16),
                "w1": w1[e].astype(BF16),
                "w2": w2[e].astype(BF16),
                "pw": pw_b,
                "b1": np.ascontiguousarray(b1[e].reshape(H // 128, 128).T),
                "b2": np.ascontiguousarray(b2[e].reshape(D // 128, 128).T),
                "pb": pb_h,
            }
        )

    kwargs = {}
    if TRACE:
        kwargs = {"trace": True, "trace_cores": list(range(N_CORES))}
    res = run_bass_kernel_spmd(nc, in_maps, core_ids=list(range(N_CORES)), **kwargs)
    LAST_EXEC_NS = res.exec_time_ns
    LAST_MEAN_EXEC_NS = res.mean_exec_time_ns

    out = np.empty((T, d_model), np.float32)
    for e in range(E):
        n = len(idx[e])
        if n:
            out[idx[e]] = res.results[e]["outT"].T[:n]
    return out.reshape(B, S, d_model), balance_loss
